# revision 3
# baseline (speedup 1.0000x reference)
"""Trainium2 Bass kernel for nn_Aggregator (GNN message-passing aggregation).

Computes, for N=16384 nodes with K=32 messages of dim D=256 each:
    out[n, :] = relu(curr_emb[n, 0, :] + sum_k alpha[n, k] * msg[n, k, :])

Strategy (memory-bound problem):
  - Data-parallel over nodes: 8 NeuronCores x 2048 nodes each.
  - Only slot 0 of curr_emb is read (host slices it; saves 496 MiB of traffic).
  - Per core, loop over 16 blocks of 128 nodes. The weighted sum runs on the
    TensorEngine as 32 block-diagonal matmuls per block accumulating into one
    PSUM tile (plus 4 identity-slice matmuls that seed PSUM with curr_emb), so
    the DMA engines, not compute, are the bottleneck:
      * moving operand of matmul g: msg of nodes 4g..4g+3 laid out as
        [128 partitions = (node%4, k), 256]
      * stationary operand: [128, 32] block-diagonal alpha built on the
        VectorEngine as mask * alpha (per-partition scalar multiply)
      * matmul g writes PSUM partitions 32*(g//8)..+32 via column tiling;
        node 128*b + p lands on PSUM partition p.
  - ScalarEngine applies relu reading PSUM, DMA stores the result.
"""

import numpy as np

N, K, D = 16384, 32, 256
N_CORES = 8
NPC = N // N_CORES  # nodes per core
P = 128  # nodes per block (= partitions)
G = P // 4  # matmul groups per block (4 nodes each)

_cache: dict = {}


def _split_excess_waits(nc, max_waits: int = 1) -> int:
    """This container's walrus rejects >1 sync-wait per instruction
    ("Too many sync wait commands"). TileContext attaches several to the
    kernel-tail drain. Hoist the excess onto NoOps injected just before the
    instruction on the same engine (sequential waits == multi-wait)."""
    import bass_rust
    from concourse import mybir

    n_split = 0
    for fn in nc.m.functions:
        for bb in fn.blocks:
            out = []
            for inst in bb.instructions:
                si = inst.sync_info
                waits = list(si.on_wait) if si is not None else []
                if len(waits) > max_waits:
                    keep = waits[-max_waits:]
                    excess = waits[:-max_waits]
                    for i0 in range(0, len(excess), max_waits):
                        nop = mybir.InstNoOp(
                            name=f"{inst.name}-wsplit{i0}", ins=[], outs=[]
                        )
                        nop.engine = inst.engine
                        nop.sync_info = bass_rust.SyncInfo(
                            on_wait=excess[i0 : i0 + max_waits], on_update=[]
                        )
                        out.append(nop)
                        n_split += 1
                    inst.sync_info = bass_rust.SyncInfo(
                        on_wait=keep, on_update=list(si.on_update)
                    )
                out.append(inst)
            bb.instructions = out
    return n_split


def build_nc(npc: int = NPC, msg_bufs: int = 3, fix_waits: bool = True, repeat: int = 1):
    """Build the single-core Bass program (replicated SPMD across 8 cores).

    repeat>1 runs the identical pass that many times back-to-back inside one
    NEFF (used only by the timing harness to difference out launch overhead).
    """
    import concourse.bass as bass
    import concourse.tile as tile
    from concourse import mybir

    f32 = mybir.dt.float32
    nb = npc // P  # node blocks

    nc = bass.Bass("TRN2", target_bir_lowering=False, debug=False, num_devices=N_CORES)

    msg_d = nc.dram_tensor("msg", [npc, K, D], f32, kind="ExternalInput").ap()
    cur_d = nc.dram_tensor("cur", [npc, D], f32, kind="ExternalInput").ap()
    at_d = nc.dram_tensor("alpha_t", [nb, P, G], f32, kind="ExternalInput").ap()
    ident_d = nc.dram_tensor("ident", [P, P], f32, kind="ExternalInput").ap()
    masks_d = nc.dram_tensor("masks", [P, 8, 32], f32, kind="ExternalInput").ap()
    out_d = nc.dram_tensor("out", [npc, D], f32, kind="ExternalOutput").ap()

    # msg viewed per block: [b, partition=(j k), group g, d]; node = 128b+4g+j
    msg_re = msg_d.rearrange("(b g j) k d -> b (j k) g d", g=G, j=4)

    with tile.TileContext(nc) as tc:
        with (
            tc.tile_pool(name="const", bufs=1) as const_pool,
            tc.tile_pool(name="msg", bufs=msg_bufs) as msg_pool,
            tc.tile_pool(name="small", bufs=3) as small_pool,
            tc.tile_pool(name="w", bufs=2) as w_pool,
            tc.tile_pool(name="o", bufs=3) as o_pool,
            tc.tile_pool(name="ps", bufs=2, space="PSUM") as ps_pool,
        ):
            ident_t = const_pool.tile([P, P], f32)
            nc.scalar.dma_start(ident_t[:], ident_d[:])
            mask_t = const_pool.tile([P, 8, 32], f32)
            nc.scalar.dma_start(mask_t[:], masks_d[:])

            for b in [b for _ in range(repeat) for b in range(nb)]:
                msg_t = msg_pool.tile([P, G, D], f32)
                nc.sync.dma_start(msg_t[:], msg_re[b])
                cur_t = small_pool.tile([P, D], f32, tag="cur")
                nc.sync.dma_start(cur_t[:], cur_d[b * P : (b + 1) * P, :])
                a_t = small_pool.tile([P, G], f32, tag="a")
                nc.sync.dma_start(a_t[:], at_d[b])

                w_t = w_pool.tile([P, G, 32], f32)
                for g in range(G):
                    nc.vector.tensor_scalar(
                        w_t[:, g, :],
                        mask_t[:, g % 8, :],
                        a_t[:, g : g + 1],
                        None,
                        op0=mybir.AluOpType.mult,
                    )

                ps_t = ps_pool.tile([P, D], f32)
                for cg in range(4):
                    # seed PSUM partitions 32cg..32cg+32 with curr_emb rows
                    nc.tensor.matmul(
                        ps_t[32 * cg : 32 * (cg + 1), :],
                        ident_t[:, 32 * cg : 32 * (cg + 1)],
                        cur_t[:],
                        start=True,
                        stop=False,
                        tile_position=(0, 32 * cg),
                    )
                    for g in range(8 * cg, 8 * cg + 8):
                        nc.tensor.matmul(
                            ps_t[32 * cg : 32 * (cg + 1), :],
                            w_t[:, g, :],
                            msg_t[:, g, :],
                            start=False,
                            stop=(g % 8 == 7),
                            tile_position=(0, 32 * cg),
                        )

                o_t = o_pool.tile([P, D], f32)
                nc.scalar.activation(
                    o_t[:], ps_t[:], mybir.ActivationFunctionType.Relu
                )
                nc.scalar.dma_start(out_d[b * P : (b + 1) * P, :], o_t[:])

    if fix_waits:
        _split_excess_waits(nc)
    return nc


def _host_prep(curr_emb, alpha, msg, npc):
    """Shard + repack host-side. Returns per-core input maps."""
    nb = npc // P
    n = npc * N_CORES
    cur = np.ascontiguousarray(curr_emb[:, 0, :], dtype=np.float32)  # [N, D]
    al = np.asarray(alpha, dtype=np.float32).reshape(n, K)  # [N, K]
    # alpha_t[core, b, 32j+k, g] = al[core*npc + 128b + 4g + j, k]
    at = al.reshape(N_CORES, nb, G, 4, K).transpose(0, 1, 3, 4, 2)
    at = np.ascontiguousarray(at).reshape(N_CORES, nb, P, G)

    ident = np.eye(P, dtype=np.float32)
    masks = np.zeros((P, 8, 32), dtype=np.float32)
    for c in range(8):
        for j in range(4):
            masks[32 * j : 32 * (j + 1), c, 4 * c + j] = 1.0

    msg = np.asarray(msg, dtype=np.float32)
    in_maps = []
    for core in range(N_CORES):
        sl = slice(core * npc, (core + 1) * npc)
        in_maps.append(
            {
                "msg": np.ascontiguousarray(msg[sl]),
                "cur": np.ascontiguousarray(cur[sl]),
                "alpha_t": at[core],
                "ident": ident,
                "masks": masks,
            }
        )
    return in_maps


def kernel(curr_emb, alpha, msg):
    from concourse.bass_utils import run_bass_kernel_spmd

    if "nc" not in _cache:
        _cache["nc"] = build_nc()
    nc = _cache["nc"]
    in_maps = _host_prep(curr_emb, alpha, msg, NPC)
    res = run_bass_kernel_spmd(nc, in_maps, list(range(N_CORES)))
    out = np.concatenate([res.results[i]["out"] for i in range(N_CORES)], axis=0)
    return out.astype(np.float32, copy=False)



# revision 11
# speedup vs baseline: 2.4156x; 2.4156x over previous
"""Trainium2 Bass kernel for nn_Aggregator (GNN message-passing aggregation).

Computes, for N=16384 nodes with K=32 messages of dim D=256 each:
    out[n, :] = relu(curr_emb[n, 0, :] + sum_k alpha[n, k] * msg[n, k, :])

Strategy (memory-bound problem):
  - Data-parallel over nodes: 8 NeuronCores x 2048 nodes each.
  - Only slot 0 of curr_emb is read (host slices it; saves 496 MiB of traffic).
  - Per core, loop over 16 blocks of 128 nodes. The weighted sum runs on the
    TensorEngine as 32 block-diagonal matmuls per block accumulating into one
    PSUM tile (plus 4 identity-slice matmuls that seed PSUM with curr_emb), so
    the DMA engines, not compute, are the bottleneck:
      * moving operand of matmul g: msg of nodes 4g..4g+3 laid out as
        [128 partitions = (node%4, k), 256]
      * stationary operand: [128, 32] block-diagonal alpha built on the
        VectorEngine as mask * alpha (per-partition scalar multiply)
      * matmul g writes PSUM partitions 32*(g//8)..+32 via column tiling;
        node 128*b + p lands on PSUM partition p.
  - ScalarEngine applies relu reading PSUM, DMA stores the result.
"""

import numpy as np

N, K, D = 16384, 32, 256
N_CORES = 8
NPC = N // N_CORES  # nodes per core
P = 128  # nodes per block (= partitions)
G = P // 4  # matmul groups per block (4 nodes each)

_cache: dict = {}


def _split_excess_waits(nc, max_waits: int = 1) -> int:
    """This container's walrus rejects >1 sync-wait per instruction
    ("Too many sync wait commands"). TileContext attaches several to the
    kernel-tail drain. Hoist the excess onto NoOps injected just before the
    instruction on the same engine (sequential waits == multi-wait)."""
    import bass_rust
    from concourse import mybir

    n_split = 0
    for fn in nc.m.functions:
        for bb in fn.blocks:
            out = []
            for inst in bb.instructions:
                si = inst.sync_info
                waits = list(si.on_wait) if si is not None else []
                if len(waits) > max_waits:
                    keep = waits[-max_waits:]
                    excess = waits[:-max_waits]
                    for i0 in range(0, len(excess), max_waits):
                        nop = mybir.InstNoOp(
                            name=f"{inst.name}-wsplit{i0}", ins=[], outs=[]
                        )
                        nop.engine = inst.engine
                        nop.sync_info = bass_rust.SyncInfo(
                            on_wait=excess[i0 : i0 + max_waits], on_update=[]
                        )
                        out.append(nop)
                        n_split += 1
                    inst.sync_info = bass_rust.SyncInfo(
                        on_wait=keep, on_update=list(si.on_update)
                    )
                out.append(inst)
            bb.instructions = out
    return n_split


def build_nc(npc: int = NPC, msg_bufs: int = 3, fix_waits: bool = True, repeat: int = 1):
    """Build the single-core Bass program (replicated SPMD across 8 cores).

    repeat>1 runs the identical pass that many times back-to-back inside one
    NEFF (used only by the timing harness to difference out launch overhead).
    """
    import concourse.bass as bass
    import concourse.tile as tile
    from concourse import mybir

    f32 = mybir.dt.float32
    nb = npc // P  # node blocks

    nc = bass.Bass("TRN2", target_bir_lowering=False, debug=False, num_devices=N_CORES)

    msg_d = nc.dram_tensor("msg", [npc, K, D], f32, kind="ExternalInput").ap()
    cur_d = nc.dram_tensor("cur", [npc, D], f32, kind="ExternalInput").ap()
    at_d = nc.dram_tensor("alpha_t", [nb, P, G], f32, kind="ExternalInput").ap()
    ident_d = nc.dram_tensor("ident", [P, P], f32, kind="ExternalInput").ap()
    masks_d = nc.dram_tensor("masks", [P, 8, 32], f32, kind="ExternalInput").ap()
    out_d = nc.dram_tensor("out", [npc, D], f32, kind="ExternalOutput").ap()

    # msg viewed per block: [b, partition=(j k), group g, d]; node = 128b+4g+j
    msg_re = msg_d.rearrange("(b g j) k d -> b (j k) g d", g=G, j=4)

    with tile.TileContext(nc) as tc:
        with (
            tc.tile_pool(name="const", bufs=1) as const_pool,
            tc.tile_pool(name="msg", bufs=msg_bufs) as msg_pool,
            tc.tile_pool(name="small", bufs=3) as small_pool,
            tc.tile_pool(name="w", bufs=2) as w_pool,
            tc.tile_pool(name="o", bufs=3) as o_pool,
            tc.tile_pool(name="ps", bufs=2, space="PSUM") as ps_pool,
        ):
            ident_t = const_pool.tile([P, P], f32)
            nc.scalar.dma_start(ident_t[:], ident_d[:])
            mask_t = const_pool.tile([P, 8, 32], f32)
            nc.scalar.dma_start(mask_t[:], masks_d[:])

            for b in [b for _ in range(repeat) for b in range(nb)]:
                msg_t = msg_pool.tile([P, G, D], f32)
                nc.sync.dma_start(msg_t[:], msg_re[b])
                cur_t = small_pool.tile([P, D], f32, tag="cur")
                nc.sync.dma_start(cur_t[:], cur_d[b * P : (b + 1) * P, :])
                a_t = small_pool.tile([P, G], f32, tag="a")
                nc.sync.dma_start(a_t[:], at_d[b])

                w_t = w_pool.tile([P, G, 32], f32)
                for g in range(G):
                    nc.vector.tensor_scalar(
                        w_t[:, g, :],
                        mask_t[:, g % 8, :],
                        a_t[:, g : g + 1],
                        None,
                        op0=mybir.AluOpType.mult,
                    )

                ps_t = ps_pool.tile([P, D], f32)
                for cg in range(4):
                    # seed PSUM partitions 32cg..32cg+32 with curr_emb rows
                    nc.tensor.matmul(
                        ps_t[32 * cg : 32 * (cg + 1), :],
                        ident_t[:, 32 * cg : 32 * (cg + 1)],
                        cur_t[:],
                        start=True,
                        stop=False,
                        tile_position=(0, 32 * cg),
                    )
                    for g in range(8 * cg, 8 * cg + 8):
                        nc.tensor.matmul(
                            ps_t[32 * cg : 32 * (cg + 1), :],
                            w_t[:, g, :],
                            msg_t[:, g, :],
                            start=False,
                            stop=(g % 8 == 7),
                            tile_position=(0, 32 * cg),
                        )

                o_t = o_pool.tile([P, D], f32)
                nc.scalar.activation(
                    o_t[:], ps_t[:], mybir.ActivationFunctionType.Relu
                )
                nc.scalar.dma_start(out_d[b * P : (b + 1) * P, :], o_t[:])

    if fix_waits:
        _split_excess_waits(nc)
    return nc


def build_nc_v2(npc: int = NPC, msg_bufs: int = 3, fix_waits: bool = True, repeat: int = 1):
    """v2: all 32 weighted-sum matmuls on the PE in float32r (1 cycle/row vs 4
    for plain fp32), dense block-diagonal weights shipped from the host (no
    on-chip weight build), msg loads split across both HWDGE queues (qSP via
    nc.sync, qAct via nc.scalar), curr_emb added to the PSUM result by one DVE
    tensor_tensor, relu on the Activation engine.

    Per-block queue balance (bytes): qSP = msg groups 0..16 (2.125 MiB) + cur
    (0.125) + out store (0.125); qAct = msg groups 17..31 (1.875) + w (0.5).
    """
    import concourse.bass as bass
    import concourse.tile as tile
    from concourse import mybir

    f32 = mybir.dt.float32
    f32r = mybir.dt.float32r
    nb = npc // P  # node blocks
    GS = 17  # msg group split between the two queues

    nc = bass.Bass("TRN2", target_bir_lowering=False, debug=False, num_devices=N_CORES)

    msg_d = nc.dram_tensor("msg", [npc, K, D], f32r, kind="ExternalInput").ap()
    cur_d = nc.dram_tensor("cur", [npc, D], f32, kind="ExternalInput").ap()
    w_d = nc.dram_tensor("w", [nb, P, G * 32], f32r, kind="ExternalInput").ap()
    out_d = nc.dram_tensor("out", [npc, D], f32, kind="ExternalOutput").ap()

    # msg viewed per block: [b, partition=(j k), group g, d]; node = 128b+4g+j
    msg_re = msg_d.rearrange("(b g j) k d -> b (j k) g d", g=G, j=4)

    with tile.TileContext(nc) as tc:
        with (
            tc.tile_pool(name="msg", bufs=msg_bufs) as msg_pool,
            tc.tile_pool(name="small", bufs=3) as small_pool,
            tc.tile_pool(name="w", bufs=3) as w_pool,
            tc.tile_pool(name="o", bufs=3) as o_pool,
            tc.tile_pool(name="ps", bufs=2, space="PSUM") as ps_pool,
        ):
            for b in [b for _ in range(repeat) for b in range(nb)]:
                msg_t = msg_pool.tile([P, G, D], f32r)
                nc.sync.dma_start(msg_t[:, 0:GS, :], msg_re[b][:, 0:GS, :])
                nc.scalar.dma_start(msg_t[:, GS:G, :], msg_re[b][:, GS:G, :])
                cur_t = small_pool.tile([P, D], f32, tag="cur")
                nc.sync.dma_start(cur_t[:], cur_d[b * P : (b + 1) * P, :])
                w_t = w_pool.tile([P, G, 32], f32r)
                nc.scalar.dma_start(w_t[:], w_d[b])

                ps_t = ps_pool.tile([P, D], f32)
                for cg in range(4):
                    for g in range(8 * cg, 8 * cg + 8):
                        nc.tensor.matmul(
                            ps_t[32 * cg : 32 * (cg + 1), :],
                            w_t[:, g, :],
                            msg_t[:, g, :],
                            start=(g % 8 == 0),
                            stop=(g % 8 == 7),
                            tile_position=(0, 32 * cg),
                        )

                o_t = o_pool.tile([P, D], f32, tag="o1")
                nc.vector.tensor_tensor(
                    o_t[:], ps_t[:], cur_t[:], op=mybir.AluOpType.add
                )
                r_t = o_pool.tile([P, D], f32, tag="o2")
                nc.scalar.activation(
                    r_t[:], o_t[:], mybir.ActivationFunctionType.Relu
                )
                nc.sync.dma_start(out_d[b * P : (b + 1) * P, :], r_t[:])

    if fix_waits:
        _split_excess_waits(nc)
    return nc


def _host_prep(curr_emb, alpha, msg, npc):
    """Shard + repack host-side. Returns per-core input maps."""
    nb = npc // P
    n = npc * N_CORES
    cur = np.ascontiguousarray(curr_emb[:, 0, :], dtype=np.float32)  # [N, D]
    al = np.asarray(alpha, dtype=np.float32).reshape(n, K)  # [N, K]
    # alpha_t[core, b, 32j+k, g] = al[core*npc + 128b + 4g + j, k]
    at = al.reshape(N_CORES, nb, G, 4, K).transpose(0, 1, 3, 4, 2)
    at = np.ascontiguousarray(at).reshape(N_CORES, nb, P, G)

    ident = np.eye(P, dtype=np.float32)
    masks = np.zeros((P, 8, 32), dtype=np.float32)
    for c in range(8):
        for j in range(4):
            masks[32 * j : 32 * (j + 1), c, 4 * c + j] = 1.0

    msg = np.asarray(msg, dtype=np.float32)
    in_maps = []
    for core in range(N_CORES):
        sl = slice(core * npc, (core + 1) * npc)
        in_maps.append(
            {
                "msg": np.ascontiguousarray(msg[sl]),
                "cur": np.ascontiguousarray(cur[sl]),
                "alpha_t": at[core],
                "ident": ident,
                "masks": masks,
            }
        )
    return in_maps


def build_nc_v3(npc: int = NPC, msg_bufs: int = 3, fix_waits: bool = True, repeat: int = 1):
    """v3: like v2, but every fp32r matmul writes PSUM partitions 0..31 (the
    fp32r ISA rejects nonzero dst partition offsets): column-group cg lands in
    free-dim slice cg of one [32, 4, D] PSUM tile, and cur/out use the matching
    node layout  node = 128*b + 32*cg + q  ->  [b, partition q, (cg d)].
    """
    import concourse.bass as bass
    import concourse.tile as tile
    from concourse import mybir

    f32 = mybir.dt.float32
    f32r = mybir.dt.float32r
    nb = npc // P  # node blocks
    GS = 17  # msg group split between the two queues

    nc = bass.Bass("TRN2", target_bir_lowering=False, debug=False, num_devices=N_CORES)

    msg_d = nc.dram_tensor("msg", [npc, K, D], f32r, kind="ExternalInput").ap()
    cur_d = nc.dram_tensor("cur", [npc, D], f32, kind="ExternalInput").ap()
    w_d = nc.dram_tensor("w", [nb, P, G * 32], f32r, kind="ExternalInput").ap()
    out_d = nc.dram_tensor("out", [npc, D], f32, kind="ExternalOutput").ap()

    # msg viewed per block: [b, partition=(j k), group g, d]; node = 128b+4g+j
    msg_re = msg_d.rearrange("(b g j) k d -> b (j k) g d", g=G, j=4)
    # cur/out viewed per block: [b, partition q, (cg d)]; node = 128b+32cg+q
    cur_re = cur_d.rearrange("(b cg q) d -> b q cg d", cg=4, q=32)
    out_re = out_d.rearrange("(b cg q) d -> b q cg d", cg=4, q=32)

    with tile.TileContext(nc) as tc:
        with (
            tc.tile_pool(name="msg", bufs=msg_bufs) as msg_pool,
            tc.tile_pool(name="small", bufs=3) as small_pool,
            tc.tile_pool(name="w", bufs=3) as w_pool,
            tc.tile_pool(name="o", bufs=3) as o_pool,
            tc.tile_pool(name="ps", bufs=2, space="PSUM") as ps_pool,
        ):
            for b in [b for _ in range(repeat) for b in range(nb)]:
                msg_t = msg_pool.tile([P, G, D], f32r)
                nc.sync.dma_start(msg_t[:, 0:GS, :], msg_re[b][:, 0:GS, :])
                nc.scalar.dma_start(msg_t[:, GS:G, :], msg_re[b][:, GS:G, :])
                cur_t = small_pool.tile([32, 4, D], f32, tag="cur")
                nc.sync.dma_start(cur_t[:], cur_re[b])
                w_t = w_pool.tile([P, G, 32], f32r)
                nc.scalar.dma_start(w_t[:], w_d[b])

                ps_t = ps_pool.tile([32, 4, D], f32)
                for cg in range(4):
                    for g in range(8 * cg, 8 * cg + 8):
                        nc.tensor.matmul(
                            ps_t[:, cg, :],
                            w_t[:, g, :],
                            msg_t[:, g, :],
                            start=(g % 8 == 0),
                            stop=(g % 8 == 7),
                        )

                o_t = o_pool.tile([32, 4, D], f32, tag="o1")
                nc.vector.tensor_tensor(
                    o_t[:], ps_t[:], cur_t[:], op=mybir.AluOpType.add
                )
                r_t = o_pool.tile([32, 4, D], f32, tag="o2")
                nc.scalar.activation(
                    r_t[:], o_t[:], mybir.ActivationFunctionType.Relu
                )
                nc.sync.dma_start(out_re[b], r_t[:])

    if fix_waits:
        _split_excess_waits(nc)
    return nc


def _host_prep_v2(curr_emb, alpha, msg, npc):
    """Shard + build dense block-diagonal PE weights host-side."""
    nb = npc // P
    n = npc * N_CORES
    cur = np.ascontiguousarray(curr_emb[:, 0, :], dtype=np.float32)  # [N, D]
    al = np.asarray(alpha, dtype=np.float32).reshape(n, K)  # [N, K]
    # at[core, b, 32j+k, g] = al[core*npc + 128b + 4g + j, k]
    at = al.reshape(N_CORES, nb, G, 4, K).transpose(0, 1, 3, 4, 2)
    at = np.ascontiguousarray(at).reshape(N_CORES, nb, P, G)

    # w[core, b, p, g, c] = at[core, b, p, g] iff c == 4*(g%8) + p//32 else 0;
    # then matmul g writes psum partition 4g+j from contraction partition 32j+k.
    pp = np.arange(P)[:, None]
    gg = np.arange(G)[None, :]
    cc = 4 * (gg % 8) + pp // 32
    w = np.zeros((N_CORES, nb, P, G, 32), np.float32)
    w[:, :, pp, gg, cc] = at
    w = w.reshape(N_CORES, nb, P, G * 32)

    msg = np.asarray(msg, dtype=np.float32)
    in_maps = []
    for core in range(N_CORES):
        sl = slice(core * npc, (core + 1) * npc)
        in_maps.append(
            {
                "msg": np.ascontiguousarray(msg[sl]),
                "cur": np.ascontiguousarray(cur[sl]),
                "w": np.ascontiguousarray(w[core]),
            }
        )
    return in_maps


def kernel(curr_emb, alpha, msg):
    from concourse.bass_utils import run_bass_kernel_spmd

    if "nc" not in _cache:
        _cache["nc"] = build_nc_v3()
    nc = _cache["nc"]
    in_maps = _host_prep_v2(curr_emb, alpha, msg, NPC)
    res = run_bass_kernel_spmd(nc, in_maps, list(range(N_CORES)))
    out = np.concatenate([res.results[i]["out"] for i in range(N_CORES)], axis=0)
    return out.astype(np.float32, copy=False)

